# revision 74
# baseline (speedup 1.0000x reference)
"""AutoCorrelation (FFT-free) kernel for 8 Trainium2 NeuronCores.

Math: the reference computes, per (b, h, e), the circular cross-correlation
corr = irfft(rfft(q) * conj(rfft(k))), then
  mean_corr[b, l] = mean_{h,e} corr          (only this is ever used)
  global_mean[l]  = mean_b mean_corr
  topk lags       = top-7 of global_mean
  weights         = softmax(mean_corr[:, topk])
  out[b,l]        = sum_k w[b,k] * v[b, (l - lag_k) % L]

Identity used: mean_corr[b, l] = (1/HE) * sum_s <q[b,(s+l)%L,:,:], k[b,s,:,:]>.
Per batch the Gram matrix G[s,t] = sum_c kT[c,s] qT[c,t] runs on the
TensorEngine (fp16 inputs, fp32 PSUM accumulation), with each s-chunk's
output columns rotated in PSUM so that afterwards
mean_corr[l] = sum_p S[p, (l+p)%L]. The channel loop is outermost so the
first (k, q) chunk pair covers 12 s-blocks (~7us) of matmul per load.

The diagonal fold: evict S to a tail-doubled fp16 SBUF tile (1/HE scale
fused), then realign the per-partition skew through DRAM -- write row p at
flat offset (LD-1)*p (the skew is expressible on the flat-addressed DRAM
side of a DMA; neither compute engines nor the compiled SBUF DMA lowering
accept a partition step of pitch+1), read back with row stride LD so
diag[p, j] = S[p, (p+j) % L], split in two column chunks on the two HWDGE
queues to pipeline. A ones-vector matmul then reduces the 128 partitions in
PSUM (512-col chunks reusing the Gram banks); global_mean accumulates both
batches' diagonals into one PSUM group on the otherwise-idle PE, and the
single-core build's top-8 reads that PSUM row directly. Batch 0's entire
fold hides under batch 1's Gram; this replaces the previous 7-level
rotate-add fold tree (~25us of serial DVE/DMA ops) with ~7us of which only
batch 1's DMA round trip is exposed.

Top-7 via the DVE max/max_index top-8 instruction. Weights: dynamic-offset
ACT/DVE element copies gather mean_corr[b, lag_k] straight from SBUF (no
SWDGE round trips), softmax per batch at partition 0 (no cross-partition
DMA hop).

Output gather-sum sum_k w_k v[(l-lag_k)%L], cost-model-balanced across
engines (PE identity-matmul tap = 2.56us, ACT stage = 5.3us, DVE fused
tap = 6.5us, DVE fused eviction slice = 0.66us):
  batch 0: taps 0-5 as w-scaled identity matmuls in PSUM + ACT stage (tap 6)
  batch 1: taps 0-4 on PE + ACT stage (tap 6) + one fused DVE tap (5)
Evictions are fused DVE adds (out = psum + acc), so the ACT/DVE accumulator
is folded in for free and PE never waits on PSUM reuse.

Sharding: batch across the 8 cores (2 per core). Only global_mean needs an
AllReduce of a [1,1536] fp32 vector.

fp16 is safe here: top-7 global_mean gap is 1.5e-3 while the fp16-input plus
fp16-S-eviction error is <6e-4 (validated against the fp32 FFT reference on
the actual seed); output tolerance is 2e-2 vs our ~7e-4.
"""

import numpy as np

B, L, H, E = 16, 1536, 8, 64
C = H * E             # 512 channels = H*E
NCORES = 8
BLOC = B // NCORES    # batches per core
NCC = C // 128        # channel chunks of 128
TOPK = 7              # int(1 * log(1536)) == 7
NJ = L // 128         # s-chunks
NLT = L // 512        # output l-tiles

PE_TAPS = (0, 1, 2, 3, 4, 5)   # batch 0 PE taps; batch 1 uses 0-4
B1_DVE_TAP = 5                 # batch 1 only: fused DVE tap
ACT_TAP = 6                    # staged by the ACT engine (both batches)
N_WARM = 48                    # PE warmup matmuls (p-state ramp during DMAs)
ECH = 896                      # first eviction/diag column chunk split

_cache = {}
DEBUG_BUILD = False


def _build(num_cores: int):
    import concourse.bass as bass
    import concourse.bacc as bacc
    import concourse.mybir as mybir
    import concourse.tile as tile

    f16 = mybir.dt.float16
    f32 = mybir.dt.float32
    u32 = mybir.dt.uint32
    PE = mybir.EngineType.PE
    ACT = mybir.EngineType.Activation
    DVE = mybir.EngineType.DVE
    MUL = mybir.AluOpType.mult
    ADD = mybir.AluOpType.add

    nc = bacc.Bacc(None)
    qT = nc.dram_tensor("qT", [BLOC, C, L], f16, kind="ExternalInput")
    kT = nc.dram_tensor("kT", [BLOC, C, L], f16, kind="ExternalInput")
    vT = nc.dram_tensor("vT", [BLOC, C, L], f16, kind="ExternalInput")
    out = nc.dram_tensor("out", [BLOC, C, L], f16, kind="ExternalOutput")
    if DEBUG_BUILD:
        dbg_gm = nc.dram_tensor("dbg_gm", [1, L], f32, kind="ExternalOutput")
        dbg_idx = nc.dram_tensor("dbg_idx", [1, 8], u32, kind="ExternalOutput")
        dbg_wr = nc.dram_tensor("dbg_wr", [BLOC, 8], f32, kind="ExternalOutput")
        dbg_mc = nc.dram_tensor("dbg_mc", [BLOC, L], f32, kind="ExternalOutput")
    ident_d = nc.inline_tensor(np.eye(128, dtype=np.float16), "identc")
    onesb_d = nc.inline_tensor(np.ones((1, 128), np.float16), "onesb")
    onesr_d = nc.inline_tensor(np.ones((128, 1), np.float16), "onesr")

    LD = L + 128          # doubled-tail S tile free size

    with tile.TileContext(nc) as tc:
        with (
            tc.tile_pool(name="sb", bufs=1) as sb,
            tc.tile_pool(name="sps", bufs=1, space="PSUM") as sps,
            tc.tile_pool(name="obp", bufs=3) as obp,
            tc.tile_pool(name="dram", bufs=1, space="DRAM") as dram,
        ):
            # ---- input loads: k/q first (Gram-critical) interleaved on
            # both HWDGE queues; the first matmul's operands (full q00 but
            # only the first 128 cols of k00) lead both queues ----
            ks = [[None] * NCC for _ in range(BLOC)]
            qs = [[None] * NCC for _ in range(BLOC)]
            k00 = sb.tile([128, L], f16, tag="k00")
            nc.sync.dma_start(k00, kT[0, 0:128, :])
            ks[0][0] = k00
            q00 = sb.tile([128, L], f16, tag="q00")
            nc.scalar.dma_start(q00, qT[0, 0:128, :])
            qs[0][0] = q00
            ident = sb.tile([128, 128], f16, tag="ident")
            nc.scalar.dma_start(ident, ident_d[:])
            for bi in range(BLOC):
                for cc in range(NCC):
                    if bi == 0 and cc == 0:
                        continue
                    kt = sb.tile([128, L], f16, tag=f"k{bi}{cc}")
                    nc.sync.dma_start(kt, kT[bi, 128 * cc:128 * (cc + 1), :])
                    ks[bi][cc] = kt
                    qt = sb.tile([128, L], f16, tag=f"q{bi}{cc}")
                    nc.scalar.dma_start(qt, qT[bi, 128 * cc:128 * (cc + 1), :])
                    qs[bi][cc] = qt
                if bi == 0:
                    onesb = sb.tile([1, 128], f16, tag="onesb")
                    nc.sync.dma_start(onesb, onesb_d[:])
                    onesr = sb.tile([128, 1], f16, tag="onesr")
                    nc.sync.dma_start(onesr, onesr_d[:])

            # PE warmup on a memset tile: long enough to run continuously
            # into the first Gram matmul so the p-state ramp completes before
            # real work starts (any idle gap resets the Tensor engine to a
            # 2x-slower state for its next 3us)
            junk = sb.tile([128, 128], f16, tag="junk")
            nc.vector.memset(junk, 0.0)
            warm = sps.tile([128, 128], f32, tag="wrm")
            for _ in range(N_WARM):
                nc.tensor.matmul(warm, junk, junk, start=True, stop=True,
                                 skip_group_check=True)

            # doubled v buffers: layout [128, NCC, 2L] so one dynamic-offset
            # AP can window all channel chunks at once
            vv = []
            for bi in range(BLOC):
                t = sb.tile([128, NCC, 2 * L], f16, tag=f"vv{bi}")
                for cc in range(NCC):
                    eng = nc.sync if cc % 2 == 0 else nc.scalar
                    eng.dma_start(t[:, cc, 0:L], vT[bi, 128 * cc:128 * (cc + 1), :])
                nc.vector.tensor_copy(t[:, :, L:2 * L], t[:, :, 0:L])
                vv.append(t)

            # ---- Gram with rotated PSUM accumulation; cc outermost so one
            # (k, q) chunk pair covers 12 s-blocks of matmul ----
            usegs = []
            for u in range(NJ):
                r = (L - 128 * u) % L
                segs = []
                t0 = 0
                while t0 < L:
                    y0 = (t0 + r) % L
                    seg = min(512 - (y0 % 512), L - t0, L - y0)
                    segs.append((t0, y0, seg))
                    t0 += seg
                usegs.append(segs)

            def gram_matmuls(bi, S, cc_range):
                for cc in cc_range:
                    for u in range(NJ):
                        for (ts_, ys_, seg) in usegs[u]:
                            nc.tensor.matmul(
                                S[:, ys_:ys_ + seg],
                                ks[bi][cc][:, 128 * u:128 * (u + 1)],
                                qs[bi][cc][:, ts_:ts_ + seg],
                                start=(u == 0 and cc == 0),
                                stop=(u == NJ - 1 and cc == NCC - 1),
                                skip_group_check=True,
                            )

            def evict_sdb(bi, S):
                # PSUM -> fp16 SBUF with the 1/HE scale fused, split across
                # ACT and DVE so both halves run concurrently; the wrapped
                # tail copy duplicates cols [0,128) at [L,L+128) so the
                # diagonal is one rectangle. The per-partition +1 element
                # skew is NOT expressible for compute engines or (in the
                # compiled DMA lowering) SBUF-side APs, so realign via DRAM:
                # write row p at flat offset (LD-1)*p (skewed -p per row),
                # read back with row stride LD -> diag[p, j] = sdb[p, p+j].
                # Rows overlap at one never-read address per pair (p<=127
                # keeps reads clear of it).
                sdb = sb.tile([128, LD], f16, tag=f"sdb{bi}")
                nc.scalar.mul(sdb[:, 0:L], S, 1.0 / C)
                nc.scalar.copy(sdb[:, L:LD], sdb[:, 0:128])
                # two independent DRAM staging tensors so the second write
                # chunk and first read-back chunk pipeline (a single tensor
                # would serialize on conservative whole-tile deps)
                skewA = dram.tile([128, LD], f16)
                skewB = dram.tile([128, LD], f16)
                ska, skb = skewA[:], skewB[:]
                nc.sync.dma_start(
                    bass.AP(ska.tensor, ska.offset, [(LD - 1, 128), (1, 896)]),
                    sdb[:, 0:896])
                nc.scalar.dma_start(
                    bass.AP(skb.tensor, skb.offset, [(LD - 1, 128), (1, LD - 768)]),
                    sdb[:, 768:LD])
                dg = sb.tile([128, L], f16, tag=f"diag{bi}")
                nc.sync.dma_start(
                    dg[:, 0:768],
                    bass.AP(ska.tensor, ska.offset, [(LD, 128), (1, 768)]))
                nc.scalar.dma_start(
                    dg[:, 768:L],
                    bass.AP(skb.tensor, skb.offset, [(LD, 128), (1, 768)]))

                def diag(ch):
                    return dg[:, 512 * ch:512 * (ch + 1)]
                return diag

            S0 = sps.tile([128, L], f32, tag="S0")
            gram_matmuls(0, S0, range(NCC))
            diag0 = evict_sdb(0, S0)

            S1 = sps.tile([128, L], f32, tag="S1")
            gram_matmuls(1, S1, range(0, 2))

            # mc0 = per-partition-reduced diag0, on the PE mid-Gram1
            # (S0's banks are free once its eviction ran)
            S0b = sps.tile([128, L], f32, tag="S0")
            for ch in range(NLT):
                nc.tensor.matmul(S0b[0:1, 512 * ch:512 * (ch + 1)], onesr,
                                 diag0(ch),
                                 start=True, stop=True, skip_group_check=True)
            mc0 = sb.tile([1, L], f32, tag="mc0")
            nc.scalar.copy(mc0, S0b[0:1, :])

            gram_matmuls(1, S1, range(2, NCC))
            diag1 = evict_sdb(1, S1)

            # gl = sum_p (diag0 + diag1): one PSUM accumulation group
            S1b = sps.tile([128, L], f32, tag="S1")
            for di, dg in enumerate((diag0, diag1)):
                for ch in range(NLT):
                    nc.tensor.matmul(S1b[0:1, 512 * ch:512 * (ch + 1)], onesr,
                                     dg(ch),
                                     start=(di == 0), stop=(di == 1),
                                     skip_group_check=True)

            S0c = sps.tile([128, L], f32, tag="S0")
            for ch in range(NLT):
                nc.tensor.matmul(S0c[0:1, 512 * ch:512 * (ch + 1)], onesr,
                                 diag1(ch),
                                 start=True, stop=True, skip_group_check=True)
            mc1 = sb.tile([1, L], f32, tag="mc1")
            nc.scalar.copy(mc1, S0c[0:1, :])

            # ---- global mean: cross-core AllReduce of gl ----
            if num_cores > 1:
                gl = sb.tile([1, L], f32, tag="gl")
                nc.scalar.copy(gl, S1b[0:1, :])
                cc_in = dram.tile([1, L], f32)
                cc_out = dram.tile([1, L], f32)
                nc.sync.dma_start(cc_in, gl)
                nc.gpsimd.collective_compute(
                    "AllReduce",
                    mybir.AluOpType.add,
                    replica_groups=[list(range(num_cores))],
                    ins=[cc_in.opt()],
                    outs=[cc_out.opt()],
                )
                gm = sb.tile([1, L], f32, tag="gm")
                nc.sync.dma_start(gm, cc_out)
            else:
                # single core: top-8 reads the PSUM accumulator directly
                gm = S1b[0:1, :]

            # ---- top-7 lags (top-8 instruction, first 7 used) ----
            vals = sb.tile([1, 8], f32, tag="vals")
            idxs = sb.tile([1, 8], u32, tag="idxs")
            nc.vector.max(vals, gm)
            nc.vector.max_index(idxs, vals, gm)
            if DEBUG_BUILD:
                nc.sync.dma_start(dbg_gm[:], gm)
                nc.sync.dma_start(dbg_idx[:], idxs)
                nc.sync.dma_start(dbg_mc[0:1, :], mc0)
                nc.sync.dma_start(dbg_mc[1:2, :], mc1)

            act_eng = nc.engines[ACT]
            dve_eng = nc.engines[DVE]
            pe_eng = nc.engines[PE]

            # PE tap offsets: x = (L - lag_k) + 512*lt
            sv_x = {}
            for k in PE_TAPS:
                rp = pe_eng.alloc_register(f"ip{k}")
                pe_eng.reg_load(rp, idxs[0:1, k:k + 1])
                ro = pe_eng.alloc_register(f"io{k}")
                pe_eng.reg_alu(ro, L, rp, mybir.AluOpType.subtract)
                for lt in range(NLT):
                    rx = pe_eng.alloc_register(f"ix{k}_{lt}")
                    pe_eng.reg_alu(rx, ro, 512 * lt, mybir.AluOpType.add)
                    sv_x[(k, lt)] = pe_eng.snap(rx, donate=True, min_val=1,
                                                max_val=L + 1024)
                pe_eng.free_register(ro)

            # DVE/ACT tap window offsets o = L - lag_k
            rp = dve_eng.alloc_register("dp5")
            dve_eng.reg_load(rp, idxs[0:1, B1_DVE_TAP:B1_DVE_TAP + 1])
            ro = dve_eng.alloc_register("do5")
            dve_eng.reg_alu(ro, L, rp, mybir.AluOpType.subtract)
            sv_dve_lt = []
            for lt in range(NLT):
                rx = dve_eng.alloc_register(f"dx{lt}")
                dve_eng.reg_alu(rx, ro, 512 * lt, mybir.AluOpType.add)
                sv_dve_lt.append(dve_eng.snap(rx, donate=True, min_val=1,
                                              max_val=L + 512 * lt))
            sv_dve = dve_eng.snap(ro, donate=True, min_val=1, max_val=L)
            rp = act_eng.alloc_register("ap6")
            act_eng.reg_load(rp, idxs[0:1, ACT_TAP:ACT_TAP + 1])
            ro = act_eng.alloc_register("ao6")
            act_eng.reg_alu(ro, L, rp, mybir.AluOpType.subtract)
            sv_act = act_eng.snap(ro, donate=True, min_val=1, max_val=L)

            # ---- weights: gather mean_corr[b, lag_k] straight from SBUF ----
            wr = []
            for bi in range(BLOC):
                wrt = sb.tile([1, 8], f32, tag=f"wr{bi}")
                wr.append(wrt)
            for k in range(TOPK):
                rg = dve_eng.alloc_register(f"dg{k}")
                dve_eng.reg_load(rg, idxs[0:1, k:k + 1])
                sv = dve_eng.snap(rg, donate=True, min_val=0, max_val=L - 1)
                nc.vector.tensor_copy(wr[0][0:1, k:k + 1],
                                      mc0[0:1, bass.ds(sv, 1)])
            for k in range(TOPK):
                rg = act_eng.alloc_register(f"ag{k}")
                act_eng.reg_load(rg, idxs[0:1, k:k + 1])
                sv = act_eng.snap(rg, donate=True, min_val=0, max_val=L - 1)
                nc.scalar.copy(wr[1][0:1, k:k + 1], mc1[0:1, bass.ds(sv, 1)])

            if DEBUG_BUILD:
                nc.sync.dma_start(dbg_wr[0:1, :], wr[0])
                nc.sync.dma_start(dbg_wr[1:2, :], wr[1])

            # softmax per batch (no max-subtraction: |mean_corr| is small and
            # fp32 exp is safe; softmax is shift-invariant)
            wbcs = []
            for bi in range(BLOC):
                ex = sb.tile([1, 8], f32, tag=f"ex{bi}")
                nc.scalar.activation(ex[0:1, 0:TOPK], wr[bi][0:1, 0:TOPK],
                                     mybir.ActivationFunctionType.Exp,
                                     bias=0.0, scale=1.0)
                sm = sb.tile([1, 1], f32, tag=f"sm{bi}")
                nc.vector.reduce_sum(sm, ex[0:1, 0:TOPK],
                                     axis=mybir.AxisListType.X)
                rs = sb.tile([1, 1], f32, tag=f"rs{bi}")
                nc.vector.reciprocal(rs, sm)
                w16 = sb.tile([1, 8], f16, tag=f"w16{bi}")
                nc.vector.tensor_scalar_mul(w16[0:1, 0:TOPK], ex[0:1, 0:TOPK],
                                            rs[0:1, 0:1])
                # broadcast to 128 partitions via ones-column matmul
                wps = sps.tile([128, 128], f32, tag="wps")
                nc.tensor.matmul(wps[:, 0:TOPK], onesb, w16[0:1, 0:TOPK],
                                 start=True, stop=True, skip_group_check=True)
                wbc = sb.tile([128, 8], f32, tag=f"wbc{bi}")
                nc.scalar.copy(wbc[:, 0:TOPK], wps[:, 0:TOPK])
                wbcs.append(wbc)

            # w-scaled identities for the PE taps
            Iw = [[None] * TOPK for _ in range(BLOC)]
            for bi in range(BLOC):
                taps = PE_TAPS if bi == 0 else PE_TAPS[:-1]
                for k in taps:
                    t = sb.tile([128, 128], f16, tag=f"iw{bi}{k}")
                    nc.vector.tensor_scalar_mul(t, ident, wbcs[bi][:, k:k + 1])
                    Iw[bi][k] = t

            # ---- weighted circular gather-sum ----
            # acc chains, chunked per channel chunk so cc0's eviction never
            # waits for later chunks: b0 = ACT stage only; b1 = ACT stage +
            # fused DVE tap (DVE chunks interleave behind b0's evictions)
            acc0 = sb.tile([128, NCC, L], f16, tag="acc0")
            for cc in range(NCC):
                nc.scalar.mul(acc0[:, cc, :],
                              vv[0][:, cc, bass.ds(sv_act, L)],
                              wbcs[0][:, ACT_TAP:ACT_TAP + 1])
            acc1s = sb.tile([128, NCC, L], f16, tag="acc1s")
            for cc in range(NCC):
                nc.scalar.mul(acc1s[:, cc, :],
                              vv[1][:, cc, bass.ds(sv_act, L)],
                              wbcs[1][:, ACT_TAP:ACT_TAP + 1])
            acc1 = sb.tile([128, NCC, L], f16, tag="acc1")
            accs = [acc0, acc1]

            # PE taps into PSUM; fused DVE eviction adds the chain acc;
            # b1's fused DVE tap chunks ride behind b0's evictions
            for bi in range(BLOC):
                taps = PE_TAPS if bi == 0 else PE_TAPS[:-1]
                cc_order = range(NCC) if bi == 0 else [0, 1, 3, 2]
                for cc in cc_order:
                    tgt = sps.tile([128, L], f32,
                                   tag=("S0" if cc % 2 == 0 else "S1"))
                    ot = obp.tile([128, L], f16, tag="ot")
                    fine = (bi == 1 and cc == NCC - 1) or (bi == 0 and cc < 1)
                    for lt in range(NLT):
                        for ki, k in enumerate(taps):
                            nc.tensor.matmul(
                                tgt[:, 512 * lt:512 * (lt + 1)],
                                Iw[bi][k],
                                vv[bi][:, cc, bass.ds(sv_x[(k, lt)], 512)],
                                start=(ki == 0),
                                stop=(ki == len(taps) - 1),
                                skip_group_check=True,
                            )
                        if fine:
                            nc.vector.scalar_tensor_tensor(
                                ot[:, 512 * lt:512 * (lt + 1)],
                                tgt[:, 512 * lt:512 * (lt + 1)],
                                1.0,
                                accs[bi][:, cc, 512 * lt:512 * (lt + 1)],
                                op0=MUL, op1=ADD)
                            eng = nc.sync if lt % 2 == 0 else nc.scalar
                            eng.dma_start(
                                out[bi, 128 * cc:128 * (cc + 1),
                                    512 * lt:512 * (lt + 1)],
                                ot[:, 512 * lt:512 * (lt + 1)],
                            )
                    if bi == 0:
                        nc.vector.scalar_tensor_tensor(
                            acc1[:, cc, :],
                            vv[1][:, cc, bass.ds(sv_dve, L)],
                            wbcs[1][:, B1_DVE_TAP:B1_DVE_TAP + 1],
                            acc1s[:, cc, :],
                            op0=MUL, op1=ADD)
                    if not fine:
                        # one coarse eviction per channel chunk: fewer DVE
                        # ops (DVE paces the output phase); stores stay
                        # sliced for DMA streaming
                        nc.vector.scalar_tensor_tensor(
                            ot, tgt, 1.0, accs[bi][:, cc, :],
                            op0=MUL, op1=ADD)
                        for lt in range(NLT):
                            eng = nc.sync if (cc * NLT + lt) % 2 == 0 else nc.scalar
                            eng.dma_start(
                                out[bi, 128 * cc:128 * (cc + 1),
                                    512 * lt:512 * (lt + 1)],
                                ot[:, 512 * lt:512 * (lt + 1)],
                            )
    nc.finalize()
    return nc


def _marshal(arr, ncores):
    # [B, L, H, E] fp32 -> per-core contiguous fp16 [BLOC, C, L]
    a = arr.reshape(B, L, C).astype(np.float16)
    a = np.ascontiguousarray(a.transpose(0, 2, 1))  # [B, C, L]
    bloc = B // ncores
    return [a[c * bloc:(c + 1) * bloc] for c in range(ncores)]


def _ensure_axon_hooks_importable():
    # some containers lack antenv.axon_hooks; run_bass_kernel_spmd imports it
    # unconditionally when tracing is requested. A None hook degrades to an
    # untraced run instead of crashing.
    import sys
    import types
    try:
        import antenv.axon_hooks  # noqa: F401
    except ModuleNotFoundError:
        try:
            import antenv
        except ModuleNotFoundError:
            return
        m = types.ModuleType("antenv.axon_hooks")
        m.get_axon_ntff_profile_hook = lambda: None
        sys.modules["antenv.axon_hooks"] = m
        antenv.axon_hooks = m


def kernel(queries, keys, values, attn_mask=None, _trace=False):
    from concourse.bass_utils import run_bass_kernel_spmd

    _ensure_axon_hooks_importable()

    nc = _cache.get("nc")
    if nc is None:
        nc = _build(NCORES)
        _cache["nc"] = nc

    qs = _marshal(np.asarray(queries, np.float32), NCORES)
    ks = _marshal(np.asarray(keys, np.float32), NCORES)
    vs = _marshal(np.asarray(values, np.float32), NCORES)
    in_maps = [{"qT": qs[c], "kT": ks[c], "vT": vs[c]} for c in range(NCORES)]

    res = run_bass_kernel_spmd(nc, in_maps, core_ids=list(range(NCORES)), trace=_trace)
    _cache["last"] = res
    o = np.concatenate([res.results[c]["out"] for c in range(NCORES)], axis=0)
    o = o.transpose(0, 2, 1).astype(np.float32)  # [B, L, C]
    return np.ascontiguousarray(o.reshape(B, L, H, E))


# revision 81
# speedup vs baseline: 1.0034x; 1.0034x over previous
"""AutoCorrelation (FFT-free) kernel for 8 Trainium2 NeuronCores.

Math: the reference computes, per (b, h, e), the circular cross-correlation
corr = irfft(rfft(q) * conj(rfft(k))), then
  mean_corr[b, l] = mean_{h,e} corr          (only this is ever used)
  global_mean[l]  = mean_b mean_corr
  topk lags       = top-7 of global_mean
  weights         = softmax(mean_corr[:, topk])
  out[b,l]        = sum_k w[b,k] * v[b, (l - lag_k) % L]

Identity used: mean_corr[b, l] = (1/HE) * sum_s <q[b,(s+l)%L,:,:], k[b,s,:,:]>.
Per batch the Gram matrix G[s,t] = sum_c kT[c,s] qT[c,t] runs on the
TensorEngine (fp16 inputs, fp32 PSUM accumulation), with each s-chunk's
output columns rotated in PSUM so that afterwards
mean_corr[l] = sum_p S[p, (l+p)%L]. The channel loop is outermost so the
first (k, q) chunk pair covers 12 s-blocks (~7us) of matmul per load.

The diagonal fold: evict S to a tail-doubled fp16 SBUF tile (1/HE scale
fused), then realign the per-partition skew through DRAM -- write row p at
flat offset (LD-1)*p (the skew is expressible on the flat-addressed DRAM
side of a DMA; neither compute engines nor the compiled SBUF DMA lowering
accept a partition step of pitch+1), read back with row stride LD so
diag[p, j] = S[p, (p+j) % L], split in two column chunks on the two HWDGE
queues to pipeline. A ones-vector matmul then reduces the 128 partitions in
PSUM (512-col chunks reusing the Gram banks); global_mean accumulates both
batches' diagonals into one PSUM group on the otherwise-idle PE, and the
single-core build's top-8 reads that PSUM row directly. Batch 0's entire
fold hides under batch 1's Gram; this replaces the previous 7-level
rotate-add fold tree (~25us of serial DVE/DMA ops) with ~7us of which only
batch 1's DMA round trip is exposed.

Top-7 via the DVE max/max_index top-8 instruction. Weights: dynamic-offset
ACT/DVE element copies gather mean_corr[b, lag_k] straight from SBUF (no
SWDGE round trips), softmax per batch at partition 0 (no cross-partition
DMA hop).

Output gather-sum sum_k w_k v[(l-lag_k)%L], cost-model-balanced across
engines (PE identity-matmul tap = 2.56us, ACT stage = 5.3us, DVE fused
tap = 6.5us, DVE fused eviction slice = 0.66us):
  batch 0: taps 0-5 as w-scaled identity matmuls in PSUM + ACT stage (tap 6)
  batch 1: taps 0-4 on PE + ACT stage (tap 6) + one fused DVE tap (5)
Evictions are fused DVE adds (out = psum + acc), so the ACT/DVE accumulator
is folded in for free and PE never waits on PSUM reuse.

Sharding: batch across the 8 cores (2 per core). Only global_mean needs an
AllReduce of a [1,1536] fp32 vector.

fp16 is safe here: top-7 global_mean gap is 1.5e-3 while the fp16-input plus
fp16-S-eviction error is <6e-4 (validated against the fp32 FFT reference on
the actual seed); output tolerance is 2e-2 vs our ~7e-4.
"""

import numpy as np

B, L, H, E = 16, 1536, 8, 64
C = H * E             # 512 channels = H*E
NCORES = 8
BLOC = B // NCORES    # batches per core
NCC = C // 128        # channel chunks of 128
TOPK = 7              # int(1 * log(1536)) == 7
NJ = L // 128         # s-chunks
NLT = L // 512        # output l-tiles

PE_TAPS = (0, 1, 2, 3, 4, 5)   # batch 0 PE taps; batch 1 uses 0-4
B1_DVE_TAP = 5                 # batch 1 only: fused DVE tap
ACT_TAP = 6                    # staged by the ACT engine (both batches)
N_WARM = 48                    # PE warmup matmuls (p-state ramp during DMAs)
ECH = 896                      # first eviction/diag column chunk split

_cache = {}
DEBUG_BUILD = False


def _build(num_cores: int):
    import concourse.bass as bass
    import concourse.bacc as bacc
    import concourse.mybir as mybir
    import concourse.tile as tile

    f16 = mybir.dt.float16
    f32 = mybir.dt.float32
    u32 = mybir.dt.uint32
    PE = mybir.EngineType.PE
    ACT = mybir.EngineType.Activation
    DVE = mybir.EngineType.DVE
    MUL = mybir.AluOpType.mult
    ADD = mybir.AluOpType.add

    nc = bacc.Bacc(None)
    qT = nc.dram_tensor("qT", [BLOC, C, L], f16, kind="ExternalInput")
    kT = nc.dram_tensor("kT", [BLOC, C, L], f16, kind="ExternalInput")
    vT = nc.dram_tensor("vT", [BLOC, C, L], f16, kind="ExternalInput")
    out = nc.dram_tensor("out", [BLOC, C, L], f16, kind="ExternalOutput")
    if DEBUG_BUILD:
        dbg_gm = nc.dram_tensor("dbg_gm", [1, L], f32, kind="ExternalOutput")
        dbg_idx = nc.dram_tensor("dbg_idx", [1, 8], u32, kind="ExternalOutput")
        dbg_wr = nc.dram_tensor("dbg_wr", [BLOC, 8], f32, kind="ExternalOutput")
        dbg_mc = nc.dram_tensor("dbg_mc", [BLOC, L], f32, kind="ExternalOutput")
    ident_d = nc.inline_tensor(np.eye(128, dtype=np.float16), "identc")
    onesb_d = nc.inline_tensor(np.ones((1, 128), np.float16), "onesb")
    onesr_d = nc.inline_tensor(np.ones((128, 1), np.float16), "onesr")

    LD = L + 128          # doubled-tail S tile free size

    with tile.TileContext(nc) as tc:
        with (
            tc.tile_pool(name="sb", bufs=1) as sb,
            tc.tile_pool(name="sps", bufs=1, space="PSUM") as sps,
            tc.tile_pool(name="obp", bufs=3) as obp,
            tc.tile_pool(name="dram", bufs=1, space="DRAM") as dram,
        ):
            # ---- input loads: k/q first (Gram-critical) interleaved on
            # both HWDGE queues; the first matmul's operands (full q00 but
            # only the first 128 cols of k00) lead both queues ----
            ks = [[None] * NCC for _ in range(BLOC)]
            qs = [[None] * NCC for _ in range(BLOC)]
            k00 = sb.tile([128, L], f16, tag="k00")
            nc.sync.dma_start(k00, kT[0, 0:128, :])
            ks[0][0] = k00
            q00 = sb.tile([128, L], f16, tag="q00")
            nc.scalar.dma_start(q00, qT[0, 0:128, :])
            qs[0][0] = q00
            ident = sb.tile([128, 128], f16, tag="ident")
            nc.scalar.dma_start(ident, ident_d[:])
            for bi in range(BLOC):
                for cc in range(NCC):
                    if bi == 0 and cc == 0:
                        continue
                    kt = sb.tile([128, L], f16, tag=f"k{bi}{cc}")
                    nc.sync.dma_start(kt, kT[bi, 128 * cc:128 * (cc + 1), :])
                    ks[bi][cc] = kt
                    qt = sb.tile([128, L], f16, tag=f"q{bi}{cc}")
                    nc.scalar.dma_start(qt, qT[bi, 128 * cc:128 * (cc + 1), :])
                    qs[bi][cc] = qt
                if bi == 0:
                    onesb = sb.tile([1, 128], f16, tag="onesb")
                    nc.sync.dma_start(onesb, onesb_d[:])
                    onesr = sb.tile([128, 1], f16, tag="onesr")
                    nc.sync.dma_start(onesr, onesr_d[:])

            # PE warmup on a memset tile: long enough to run continuously
            # into the first Gram matmul so the p-state ramp completes before
            # real work starts (any idle gap resets the Tensor engine to a
            # 2x-slower state for its next 3us)
            junk = sb.tile([128, 128], f16, tag="junk")
            nc.vector.memset(junk, 0.0)
            warm = sps.tile([128, 128], f32, tag="wrm")
            for _ in range(N_WARM):
                nc.tensor.matmul(warm, junk, junk, start=True, stop=True,
                                 skip_group_check=True)

            # doubled v buffers: layout [128, NCC, 2L] so one dynamic-offset
            # AP can window all channel chunks at once
            vv = []
            for bi in range(BLOC):
                t = sb.tile([128, NCC, 2 * L], f16, tag=f"vv{bi}")
                for cc in range(NCC):
                    eng = nc.sync if cc % 2 == 0 else nc.scalar
                    eng.dma_start(t[:, cc, 0:L], vT[bi, 128 * cc:128 * (cc + 1), :])
                nc.vector.tensor_copy(t[:, :, L:2 * L], t[:, :, 0:L])
                vv.append(t)

            # ---- Gram with rotated PSUM accumulation; cc outermost so one
            # (k, q) chunk pair covers 12 s-blocks of matmul ----
            usegs = []
            for u in range(NJ):
                r = (L - 128 * u) % L
                segs = []
                t0 = 0
                while t0 < L:
                    y0 = (t0 + r) % L
                    seg = min(512 - (y0 % 512), L - t0, L - y0)
                    segs.append((t0, y0, seg))
                    t0 += seg
                usegs.append(segs)

            def gram_matmuls(bi, S, cc_range):
                for cc in cc_range:
                    for u in range(NJ):
                        for (ts_, ys_, seg) in usegs[u]:
                            nc.tensor.matmul(
                                S[:, ys_:ys_ + seg],
                                ks[bi][cc][:, 128 * u:128 * (u + 1)],
                                qs[bi][cc][:, ts_:ts_ + seg],
                                start=(u == 0 and cc == 0),
                                stop=(u == NJ - 1 and cc == NCC - 1),
                                skip_group_check=True,
                            )

            def evict_sdb(bi, S):
                # PSUM -> fp16 SBUF with the 1/HE scale fused, split across
                # ACT and DVE so both halves run concurrently; the wrapped
                # tail copy duplicates cols [0,128) at [L,L+128) so the
                # diagonal is one rectangle. The per-partition +1 element
                # skew is NOT expressible for compute engines or (in the
                # compiled DMA lowering) SBUF-side APs, so realign via DRAM:
                # write row p at flat offset (LD-1)*p (skewed -p per row),
                # read back with row stride LD -> diag[p, j] = sdb[p, p+j].
                # Rows overlap at one never-read address per pair (p<=127
                # keeps reads clear of it).
                sdb = sb.tile([128, LD], f16, tag=f"sdb{bi}")
                nc.scalar.mul(sdb[:, 0:L], S, 1.0 / C)
                nc.scalar.copy(sdb[:, L:LD], sdb[:, 0:128])
                # two independent DRAM staging tensors so the second write
                # chunk and first read-back chunk pipeline (a single tensor
                # would serialize on conservative whole-tile deps)
                skewA = dram.tile([128, LD], f16)
                skewB = dram.tile([128, LD], f16)
                ska, skb = skewA[:], skewB[:]
                nc.scalar.dma_start(
                    bass.AP(ska.tensor, ska.offset, [(LD - 1, 128), (1, 896)]),
                    sdb[:, 0:896])
                nc.sync.dma_start(
                    bass.AP(skb.tensor, skb.offset, [(LD - 1, 128), (1, LD - 768)]),
                    sdb[:, 768:LD])
                dg = sb.tile([128, L], f16, tag=f"diag{bi}")
                nc.scalar.dma_start(
                    dg[:, 0:768],
                    bass.AP(ska.tensor, ska.offset, [(LD, 128), (1, 768)]))
                nc.sync.dma_start(
                    dg[:, 768:L],
                    bass.AP(skb.tensor, skb.offset, [(LD, 128), (1, 768)]))

                def diag(ch):
                    return dg[:, 512 * ch:512 * (ch + 1)]
                return diag

            S0 = sps.tile([128, L], f32, tag="S0")
            gram_matmuls(0, S0, range(NCC))
            diag0 = evict_sdb(0, S0)

            S1 = sps.tile([128, L], f32, tag="S1")
            gram_matmuls(1, S1, range(0, 2))

            # mc0 = per-partition-reduced diag0, on the PE mid-Gram1
            # (S0's banks are free once its eviction ran)
            S0b = sps.tile([128, L], f32, tag="S0")
            for ch in range(NLT):
                nc.tensor.matmul(S0b[0:1, 512 * ch:512 * (ch + 1)], onesr,
                                 diag0(ch),
                                 start=True, stop=True, skip_group_check=True)
            mc0 = sb.tile([1, L], f32, tag="mc0")
            nc.scalar.copy(mc0, S0b[0:1, :])

            gram_matmuls(1, S1, range(2, NCC))
            diag1 = evict_sdb(1, S1)

            # gl = sum_p (diag0 + diag1): one PSUM accumulation group
            S1b = sps.tile([128, L], f32, tag="S1")
            for di, dg in enumerate((diag0, diag1)):
                for ch in range(NLT):
                    nc.tensor.matmul(S1b[0:1, 512 * ch:512 * (ch + 1)], onesr,
                                     dg(ch),
                                     start=(di == 0), stop=(di == 1),
                                     skip_group_check=True)

            S0c = sps.tile([128, L], f32, tag="S0")
            for ch in range(NLT):
                nc.tensor.matmul(S0c[0:1, 512 * ch:512 * (ch + 1)], onesr,
                                 diag1(ch),
                                 start=True, stop=True, skip_group_check=True)
            mc1 = sb.tile([1, L], f32, tag="mc1")
            nc.scalar.copy(mc1, S0c[0:1, :])

            # ---- global mean: cross-core AllReduce of gl ----
            if num_cores > 1:
                gl = sb.tile([1, L], f32, tag="gl")
                nc.scalar.copy(gl, S1b[0:1, :])
                cc_in = dram.tile([1, L], f32)
                cc_out = dram.tile([1, L], f32)
                nc.sync.dma_start(cc_in, gl)
                nc.gpsimd.collective_compute(
                    "AllReduce",
                    mybir.AluOpType.add,
                    replica_groups=[list(range(num_cores))],
                    ins=[cc_in.opt()],
                    outs=[cc_out.opt()],
                )
                gm = sb.tile([1, L], f32, tag="gm")
                nc.sync.dma_start(gm, cc_out)
            else:
                # single core: top-8 reads the PSUM accumulator directly
                gm = S1b[0:1, :]

            # ---- top-7 lags (top-8 instruction, first 7 used) ----
            vals = sb.tile([1, 8], f32, tag="vals")
            idxs = sb.tile([1, 8], u32, tag="idxs")
            nc.vector.max(vals, gm)
            nc.vector.max_index(idxs, vals, gm)
            if DEBUG_BUILD:
                nc.sync.dma_start(dbg_gm[:], gm)
                nc.sync.dma_start(dbg_idx[:], idxs)
                nc.sync.dma_start(dbg_mc[0:1, :], mc0)
                nc.sync.dma_start(dbg_mc[1:2, :], mc1)

            act_eng = nc.engines[ACT]
            dve_eng = nc.engines[DVE]
            pe_eng = nc.engines[PE]

            # PE tap offsets: x = (L - lag_k) + 512*lt
            sv_x = {}
            for k in PE_TAPS:
                rp = pe_eng.alloc_register(f"ip{k}")
                pe_eng.reg_load(rp, idxs[0:1, k:k + 1])
                ro = pe_eng.alloc_register(f"io{k}")
                pe_eng.reg_alu(ro, L, rp, mybir.AluOpType.subtract)
                for lt in range(NLT):
                    rx = pe_eng.alloc_register(f"ix{k}_{lt}")
                    pe_eng.reg_alu(rx, ro, 512 * lt, mybir.AluOpType.add)
                    sv_x[(k, lt)] = pe_eng.snap(rx, donate=True, min_val=1,
                                                max_val=L + 1024)
                pe_eng.free_register(ro)

            # DVE/ACT tap window offsets o = L - lag_k
            rp = dve_eng.alloc_register("dp5")
            dve_eng.reg_load(rp, idxs[0:1, B1_DVE_TAP:B1_DVE_TAP + 1])
            ro = dve_eng.alloc_register("do5")
            dve_eng.reg_alu(ro, L, rp, mybir.AluOpType.subtract)
            sv_dve_lt = []
            for lt in range(NLT):
                rx = dve_eng.alloc_register(f"dx{lt}")
                dve_eng.reg_alu(rx, ro, 512 * lt, mybir.AluOpType.add)
                sv_dve_lt.append(dve_eng.snap(rx, donate=True, min_val=1,
                                              max_val=L + 512 * lt))
            sv_dve = dve_eng.snap(ro, donate=True, min_val=1, max_val=L)
            rp = act_eng.alloc_register("ap6")
            act_eng.reg_load(rp, idxs[0:1, ACT_TAP:ACT_TAP + 1])
            ro = act_eng.alloc_register("ao6")
            act_eng.reg_alu(ro, L, rp, mybir.AluOpType.subtract)
            sv_act = act_eng.snap(ro, donate=True, min_val=1, max_val=L)

            # ---- weights: gather mean_corr[b, lag_k] straight from SBUF ----
            wr = []
            for bi in range(BLOC):
                wrt = sb.tile([1, 8], f32, tag=f"wr{bi}")
                wr.append(wrt)
            for k in range(TOPK):
                rg = dve_eng.alloc_register(f"dg{k}")
                dve_eng.reg_load(rg, idxs[0:1, k:k + 1])
                sv = dve_eng.snap(rg, donate=True, min_val=0, max_val=L - 1)
                nc.vector.tensor_copy(wr[0][0:1, k:k + 1],
                                      mc0[0:1, bass.ds(sv, 1)])
            for k in range(TOPK):
                rg = act_eng.alloc_register(f"ag{k}")
                act_eng.reg_load(rg, idxs[0:1, k:k + 1])
                sv = act_eng.snap(rg, donate=True, min_val=0, max_val=L - 1)
                nc.scalar.copy(wr[1][0:1, k:k + 1], mc1[0:1, bass.ds(sv, 1)])

            if DEBUG_BUILD:
                nc.sync.dma_start(dbg_wr[0:1, :], wr[0])
                nc.sync.dma_start(dbg_wr[1:2, :], wr[1])

            # softmax per batch (no max-subtraction: |mean_corr| is small and
            # fp32 exp is safe; softmax is shift-invariant)
            wbcs = []
            for bi in range(BLOC):
                ex = sb.tile([1, 8], f32, tag=f"ex{bi}")
                nc.scalar.activation(ex[0:1, 0:TOPK], wr[bi][0:1, 0:TOPK],
                                     mybir.ActivationFunctionType.Exp,
                                     bias=0.0, scale=1.0)
                sm = sb.tile([1, 1], f32, tag=f"sm{bi}")
                nc.vector.reduce_sum(sm, ex[0:1, 0:TOPK],
                                     axis=mybir.AxisListType.X)
                rs = sb.tile([1, 1], f32, tag=f"rs{bi}")
                nc.vector.reciprocal(rs, sm)
                w16 = sb.tile([1, 8], f16, tag=f"w16{bi}")
                nc.vector.tensor_scalar_mul(w16[0:1, 0:TOPK], ex[0:1, 0:TOPK],
                                            rs[0:1, 0:1])
                # broadcast to 128 partitions via ones-column matmul
                wps = sps.tile([128, 128], f32, tag="wps")
                nc.tensor.matmul(wps[:, 0:TOPK], onesb, w16[0:1, 0:TOPK],
                                 start=True, stop=True, skip_group_check=True)
                wbc = sb.tile([128, 8], f32, tag=f"wbc{bi}")
                nc.scalar.copy(wbc[:, 0:TOPK], wps[:, 0:TOPK])
                wbcs.append(wbc)

            # w-scaled identities for the PE taps
            Iw = [[None] * TOPK for _ in range(BLOC)]
            for bi in range(BLOC):
                taps = PE_TAPS if bi == 0 else PE_TAPS[:-1]
                for k in taps:
                    t = sb.tile([128, 128], f16, tag=f"iw{bi}{k}")
                    nc.vector.tensor_scalar_mul(t, ident, wbcs[bi][:, k:k + 1])
                    Iw[bi][k] = t

            # ---- weighted circular gather-sum ----
            # acc chains, chunked per channel chunk so cc0's eviction never
            # waits for later chunks: b0 = ACT stage only; b1 = ACT stage +
            # fused DVE tap (DVE chunks interleave behind b0's evictions)
            acc0 = sb.tile([128, NCC, L], f16, tag="acc0")
            for cc in range(NCC):
                nc.scalar.mul(acc0[:, cc, :],
                              vv[0][:, cc, bass.ds(sv_act, L)],
                              wbcs[0][:, ACT_TAP:ACT_TAP + 1])
            acc1s = sb.tile([128, NCC, L], f16, tag="acc1s")
            for cc in range(NCC):
                nc.scalar.mul(acc1s[:, cc, :],
                              vv[1][:, cc, bass.ds(sv_act, L)],
                              wbcs[1][:, ACT_TAP:ACT_TAP + 1])
            acc1 = sb.tile([128, NCC, L], f16, tag="acc1")
            accs = [acc0, acc1]

            # PE taps into PSUM; fused DVE eviction adds the chain acc;
            # b1's fused DVE tap chunks ride behind b0's evictions
            for bi in range(BLOC):
                taps = PE_TAPS if bi == 0 else PE_TAPS[:-1]
                cc_order = range(NCC) if bi == 0 else [0, 1, 3, 2]
                for cc in cc_order:
                    tgt = sps.tile([128, L], f32,
                                   tag=("S0" if cc % 2 == 0 else "S1"))
                    ot = obp.tile([128, L], f16, tag="ot")
                    fine = (bi == 1 and cc == NCC - 1) or (bi == 0 and cc < 1)
                    for lt in range(NLT):
                        for ki, k in enumerate(taps):
                            nc.tensor.matmul(
                                tgt[:, 512 * lt:512 * (lt + 1)],
                                Iw[bi][k],
                                vv[bi][:, cc, bass.ds(sv_x[(k, lt)], 512)],
                                start=(ki == 0),
                                stop=(ki == len(taps) - 1),
                                skip_group_check=True,
                            )
                        if fine:
                            nc.vector.scalar_tensor_tensor(
                                ot[:, 512 * lt:512 * (lt + 1)],
                                tgt[:, 512 * lt:512 * (lt + 1)],
                                1.0,
                                accs[bi][:, cc, 512 * lt:512 * (lt + 1)],
                                op0=MUL, op1=ADD)
                            eng = nc.sync if lt % 2 == 0 else nc.scalar
                            eng.dma_start(
                                out[bi, 128 * cc:128 * (cc + 1),
                                    512 * lt:512 * (lt + 1)],
                                ot[:, 512 * lt:512 * (lt + 1)],
                            )
                    if bi == 0:
                        nc.vector.scalar_tensor_tensor(
                            acc1[:, cc, :],
                            vv[1][:, cc, bass.ds(sv_dve, L)],
                            wbcs[1][:, B1_DVE_TAP:B1_DVE_TAP + 1],
                            acc1s[:, cc, :],
                            op0=MUL, op1=ADD)
                    if not fine:
                        # one coarse eviction per channel chunk: fewer DVE
                        # ops (DVE paces the output phase); stores stay
                        # sliced for DMA streaming
                        nc.vector.scalar_tensor_tensor(
                            ot, tgt, 1.0, accs[bi][:, cc, :],
                            op0=MUL, op1=ADD)
                        for lt in range(NLT):
                            eng = nc.sync if (cc * NLT + lt) % 2 == 0 else nc.scalar
                            eng.dma_start(
                                out[bi, 128 * cc:128 * (cc + 1),
                                    512 * lt:512 * (lt + 1)],
                                ot[:, 512 * lt:512 * (lt + 1)],
                            )
    nc.finalize()
    return nc


def _marshal(arr, ncores):
    # [B, L, H, E] fp32 -> per-core contiguous fp16 [BLOC, C, L]
    a = arr.reshape(B, L, C).astype(np.float16)
    a = np.ascontiguousarray(a.transpose(0, 2, 1))  # [B, C, L]
    bloc = B // ncores
    return [a[c * bloc:(c + 1) * bloc] for c in range(ncores)]


def _ensure_axon_hooks_importable():
    # some containers lack antenv.axon_hooks; run_bass_kernel_spmd imports it
    # unconditionally when tracing is requested. A None hook degrades to an
    # untraced run instead of crashing.
    import sys
    import types
    try:
        import antenv.axon_hooks  # noqa: F401
    except ModuleNotFoundError:
        try:
            import antenv
        except ModuleNotFoundError:
            return
        m = types.ModuleType("antenv.axon_hooks")
        m.get_axon_ntff_profile_hook = lambda: None
        sys.modules["antenv.axon_hooks"] = m
        antenv.axon_hooks = m


def kernel(queries, keys, values, attn_mask=None, _trace=False):
    from concourse.bass_utils import run_bass_kernel_spmd

    _ensure_axon_hooks_importable()

    nc = _cache.get("nc")
    if nc is None:
        nc = _build(NCORES)
        _cache["nc"] = nc

    qs = _marshal(np.asarray(queries, np.float32), NCORES)
    ks = _marshal(np.asarray(keys, np.float32), NCORES)
    vs = _marshal(np.asarray(values, np.float32), NCORES)
    in_maps = [{"qT": qs[c], "kT": ks[c], "vT": vs[c]} for c in range(NCORES)]

    res = run_bass_kernel_spmd(nc, in_maps, core_ids=list(range(NCORES)), trace=_trace)
    _cache["last"] = res
    o = np.concatenate([res.results[c]["out"] for c in range(NCORES)], axis=0)
    o = o.transpose(0, 2, 1).astype(np.float32)  # [B, L, C]
    return np.ascontiguousarray(o.reshape(B, L, H, E))
